# revision 1
# baseline (speedup 1.0000x reference)
"""CoxPHLoss (Efron ties) Trainium2 kernel.

Host does layout only: per-column stable sort permutation by descending
duration (index-space op) + sentinel padding; all floating-point loss
arithmetic runs on 8 NeuronCores as a streaming raw-Bass pipeline:
  exp -> cumsum (tensor_tensor_scan, two-level carry fixup via PE matmul)
  -> segmented scans keyed on duration-run resets -> reverse broadcast
  scans for per-tie-group aggregates -> per-event Efron term
  log(R - (m/D)*S) -> masked reductions -> per-column losses.
Columns (B*E = 128) are sharded 16 per core; the final masked mean over
the 128 per-column losses is the host-side "gather/unshard" step.
"""
import sys

sys.path.insert(0, "/opt/trn_rl_repo")

import numpy as np

B, N, E = 16, 32768, 8
NCORES = 8
COLS = B * E              # 128 independent (b, i) columns
CPC = COLS // NCORES      # 16 columns per core
PAD = 128                 # > max run length of equal durations in a column
CH = 16                   # chunks per column
V = N // CH               # 2048 valid samples per chunk
T = V + 2 * PAD           # 2304 tile width
L = N + 2 * PAD           # 33024 padded column length
PASSES = 2                # 8 cols * 16 chunks = 128 partitions per pass
CPP = CPC // PASSES       # 8 columns per pass

_CACHE = {}


def _host_prep(logh, events, durations):
    lh = np.ascontiguousarray(logh.transpose(0, 2, 1).reshape(COLS, N))
    ev = np.ascontiguousarray(events.transpose(0, 2, 1).reshape(COLS, N))
    du = np.ascontiguousarray(durations.transpose(0, 2, 1).reshape(COLS, N))
    order = np.argsort(-du, axis=1, kind="stable")
    lh_s = np.take_along_axis(lh, order, 1).astype(np.float32)
    ev_s = np.take_along_axis(ev, order, 1).astype(np.float32)
    du_s = np.take_along_axis(du, order, 1).astype(np.float32)

    lh_p = np.zeros((COLS, L), np.float32)
    ev_p = np.zeros((COLS, L), np.float32)
    du_p = np.empty((COLS, L), np.float32)
    du_p[:, :PAD] = -2.0
    du_p[:, PAD + N:] = -1.0
    lh_p[:, PAD:PAD + N] = lh_s
    ev_p[:, PAD:PAD + N] = ev_s
    du_p[:, PAD:PAD + N] = du_s

    # constant matrices for the on-device prefix/combine matmuls
    lmat = np.zeros((128, 128), np.float32)   # G[p] = sum_{k<=p, same col} ct[k]
    for p in range(128):
        c0 = (p // CH) * CH
        lmat[c0:p + 1, p] = 1.0
    bmat = np.zeros((128, CPP), np.float32)   # colsum[m] = sum over col m's chunks
    for k in range(128):
        bmat[k, k // CH] = 1.0
    return lh_p, du_p, ev_p, lmat, bmat


def pysim_core(lh_p, du_p, ev_p):
    """Numpy mirror of the device pipeline for one core's [CPC, L] arrays."""
    losses = np.zeros(CPC, np.float32)
    for g in range(PASSES):
        lh_t = np.zeros((128, T), np.float32)
        du_t = np.zeros((128, T), np.float32)
        ev_t = np.zeros((128, T), np.float32)
        for p in range(128):
            c, k = g * CPP + p // CH, p % CH
            s = k * V
            lh_t[p], du_t[p], ev_t[p] = lh_p[c, s:s + T], du_p[c, s:s + T], ev_p[c, s:s + T]
        cm = np.zeros((128, T + 1), np.float32)
        cm[:, 1:T] = (du_t[:, 1:] == du_t[:, :-1]).astype(np.float32)
        rsp1 = 1.0 - cm[:, 1:T + 1]
        elh_sum = (ev_t[:, PAD:PAD + V] * lh_t[:, PAD:PAD + V]).sum(1, dtype=np.float32)
        e_sum = ev_t[:, PAD:PAD + V].sum(1, dtype=np.float32)
        w = np.exp(lh_t)
        cwl = np.cumsum(w, 1, dtype=np.float32)
        a = cwl[:, PAD + V - 1]
        b = cwl[:, PAD - 1]
        ct = a - b
        G = np.array([ct[(p // CH) * CH:p + 1].sum() for p in range(128)], np.float32)
        C = G - a
        cw = cwl + C[:, None]
        ew = ev_t * w
        cev = ev_t * cw

        def fscan(d0, d1, op):
            out = np.empty((128, T), np.float32)
            st = np.zeros(128, np.float32)
            for t in range(T):
                st = op(d0[:, t] * st, d1[:, t])
                out[:, t] = st
            return out

        mcnt = fscan(cm[:, :T], ev_t, np.add)
        sfwd = fscan(cm[:, :T], ew, np.add)
        fmax = fscan(cm[:, :T], cev, np.maximum)

        def rscan(src):
            out = np.empty((128, T), np.float32)
            st = np.zeros(128, np.float32)
            for t in range(T - 1, -1, -1):
                st = cm[:, t + 1] * st + src[:, t]
                out[:, t] = st
            return out

        Dbc = rscan(mcnt * rsp1)
        Sbc = rscan(sfwd * rsp1)
        Rbc = rscan(fmax * rsp1)
        VS = slice(PAD, PAD + V)
        m = mcnt[:, VS] - ev_t[:, VS]
        recD = (1.0 / np.maximum(Dbc[:, VS], 1.0)).astype(np.float32)
        arg = Rbc[:, VS] - m * recD * Sbc[:, VS]
        lsl = np.log(np.maximum(arg, 1e-30), dtype=np.float32)
        ls_sum = (lsl * ev_t[:, VS]).sum(1, dtype=np.float32)
        pp = np.stack([ls_sum, elh_sum, e_sum], 1)
        for mcol in range(CPP):
            cs = pp[mcol * CH:(mcol + 1) * CH].sum(0, dtype=np.float32)
            losses[g * CPP + mcol] = (cs[0] - cs[1]) / cs[2]
    return losses


def _build_bass():
    import concourse.bass as bass
    from concourse import mybir

    A = mybir.AluOpType
    F = mybir.ActivationFunctionType
    f32 = mybir.dt.float32
    nc = bass.Bass()

    lh_d = nc.dram_tensor("lh", [CPC, L], f32, kind="ExternalInput")
    du_d = nc.dram_tensor("du", [CPC, L], f32, kind="ExternalInput")
    ev_d = nc.dram_tensor("ev", [CPC, L], f32, kind="ExternalInput")
    lm_d = nc.dram_tensor("lmat", [128, 128], f32, kind="ExternalInput")
    bm_d = nc.dram_tensor("bmat", [128, CPP], f32, kind="ExternalInput")
    ls_d = nc.dram_tensor("loss", [CPC], f32, kind="ExternalOutput")

    import contextlib

    st = contextlib.ExitStack()

    def sb(shape, name):
        return st.enter_context(nc.sbuf_tensor(name, shape, f32))

    # work slabs
    S = {n: sb([128, T], "slab_" + n) for n in
         ["w", "rsp1", "cwl", "cw", "ew", "cev", "mc", "sf", "fm", "x1", "x2", "x3"]}
    S["cm"] = sb([128, T + 1], "slab_cm")
    lh_t = [sb([128, T], f"lh_t{i}") for i in range(2)]
    du_t = [sb([128, T], f"du_t{i}") for i in range(2)]
    ev_t = [sb([128, T], f"ev_t{i}") for i in range(2)]
    lm_t = sb([128, 128], "lm_t")
    bm_t = sb([128, CPP], "bm_t")
    sm = {n: sb([128, 1], "sm_" + n) for n in ["a", "b", "ct", "C", "ones", "trash"]}
    pp_t = sb([128, 3], "pp_t")
    cs_t = sb([128, 3], "cs_t")
    loss_t = sb([128, 1], "loss_t")
    psG = st.enter_context(nc.psum_tensor("psG", [128, 1], f32))
    ps2 = st.enter_context(nc.psum_tensor("ps2", [128, 3], f32))

    sems = {n: st.enter_context(nc.semaphore(n))
            for n in ["sv", "sa", "sp", "din0", "din1", "dout"]}

    with st:
        with nc.Block() as blk:
            eng_of = {"v": "vector", "a": "scalar", "p": "tensor"}
            cnt = {"v": 0, "a": 0, "p": 0, "din0": 0, "din1": 0, "dout": 0}
            waited = {}
            track = {}  # id(handle) -> {"w": (kind, tick), "r": [...]}

            def rec(h):
                return track.setdefault(id(h), {"w": None, "r": []})

            def dep_waits(eng, reads, writes, serialize=False):
                need = {}
                if serialize:
                    for k in ("v", "a", "p"):
                        if k != eng and cnt[k] > 0:
                            need[k] = cnt[k]
                for h in reads:
                    r = rec(h)
                    if r["w"]:
                        k, t = r["w"]
                        if k != eng:
                            need[k] = max(need.get(k, 0), t)
                for h in writes:
                    r = rec(h)
                    if r["w"]:
                        k, t = r["w"]
                        if k != eng:
                            need[k] = max(need.get(k, 0), t)
                    for k, t in r["r"]:
                        if k != eng:
                            need[k] = max(need.get(k, 0), t)
                out = []
                for k, t in need.items():
                    semname = k if k.startswith("d") else {"v": "sv", "a": "sa", "p": "sp"}[k]
                    val = t * 16 if k.startswith("d") else t
                    if waited.get((eng, semname), -1) < val:
                        out.append((semname, val))
                        waited[(eng, semname)] = val
                return out

            def emit(eng, fn, reads=(), writes=(), scan=False):
                ws = dep_waits(eng, reads, writes, serialize=True)
                tick = cnt[eng] + 1

                def body(proxy):
                    for semname, val in ws:
                        proxy.wait_ge(sems[semname], val)
                    inst = fn(proxy)
                    if scan:
                        proxy.tensor_copy(sm["trash"][:, :], sm["ones"][:, :]).then_inc(sems["sv"], 1)
                    else:
                        inst.then_inc(sems[{"v": "sv", "a": "sa", "p": "sp"}[eng]], 1)

                getattr(blk, eng_of[eng])(body)
                cnt[eng] = tick
                for h in reads:
                    rec(h)["r"].append((eng, tick))
                for h in writes:
                    track[id(h)] = {"w": (eng, tick), "r": []}

            def emit_dma(semname, out_ap, in_ap, reads=(), writes=()):
                ws = dep_waits(semname, reads, writes)
                cnt[semname] += 1
                tick = cnt[semname]

                def body(proxy):
                    for sn, val in ws:
                        proxy.wait_ge(sems[sn], val)
                    proxy.dma_start(out=out_ap, in_=in_ap).then_inc(sems[semname], 16)

                blk.sync(body)
                for h in reads:
                    rec(h)["r"].append((semname, tick))
                for h in writes:
                    track[id(h)] = {"w": (semname, tick), "r": []}

            def matmul_fn(proxy, out, lhsT, rhs):
                try:
                    return proxy.matmul(out, lhsT, rhs, start=True, stop=True)
                except TypeError:
                    return proxy.matmul(contextlib.ExitStack(), out, lhsT, rhs, start=True, stop=True)

            # constant loads + ones init
            emit_dma("din0", lm_t[:, :], lm_d[:, :], writes=[lm_t])
            emit_dma("din0", bm_t[:, :], bm_d[:, :], writes=[bm_t])
            emit("v", lambda v: v.memset(sm["ones"][:, :], 1.0), writes=[sm["ones"]])

            VS = np.s_[:, PAD:PAD + V]

            for g in range(PASSES):
                dsem = f"din{g}"
                for arr_d, arr_t in ((lh_d, lh_t[g]), (du_d, du_t[g]), (ev_d, ev_t[g])):
                    src = bass.AP(tensor=arr_d[:, :].tensor, offset=g * CPP * L,
                                  ap=[[L, CPP], [V, CH], [1, T]])
                    emit_dma(dsem, arr_t[:, :], src, writes=[arr_t])

                lh, du, ev = lh_t[g], du_t[g], ev_t[g]
                cm, rsp1 = S["cm"], S["rsp1"]
                # run masks
                emit("v", lambda v: v.tensor_tensor(out=cm[:, 1:T], in0=du[:, 1:T], in1=du[:, 0:T - 1], op=A.is_equal),
                     reads=[du], writes=[cm])
                emit("v", lambda v: v.memset(cm[:, 0:1], 0.0), writes=[cm], reads=[cm])
                emit("v", lambda v: v.memset(cm[:, T:T + 1], 0.0), writes=[cm], reads=[cm])
                emit("v", lambda v: v.tensor_scalar(out=rsp1[:, :], in0=cm[:, 1:T + 1], scalar1=-1.0, scalar2=1.0,
                                                    op0=A.mult, op1=A.add), reads=[cm], writes=[rsp1])
                # plain sums
                emit("v", lambda v: v.tensor_mul(out=S["x1"][VS], in0=ev[VS], in1=lh[VS]),
                     reads=[ev, lh], writes=[S["x1"]])
                emit("v", lambda v: v.tensor_reduce(out=pp_t[:, 1:2], in_=S["x1"][VS], axis=mybir.AxisListType.X, op=A.add),
                     reads=[S["x1"]], writes=[pp_t])
                emit("v", lambda v: v.tensor_reduce(out=pp_t[:, 2:3], in_=ev[VS], axis=mybir.AxisListType.X, op=A.add),
                     reads=[ev], writes=[pp_t])
                # w, cumsum + carry fixup
                emit("a", lambda a_: a_.activation(S["w"][:, :], lh[:, :], F.Exp), reads=[lh], writes=[S["w"]])
                emit("v", lambda v: v.tensor_tensor_scan(out=S["cwl"][:, :], data0=sm["ones"][:, :].broadcast_to([128, T]),
                                                         data1=S["w"][:, :], initial=0.0, op0=A.mult, op1=A.add),
                     reads=[S["w"], sm["ones"]], writes=[S["cwl"]], scan=True)
                emit("a", lambda a_: a_.copy(sm["a"][:, :], S["cwl"][:, PAD + V - 1:PAD + V]), reads=[S["cwl"]], writes=[sm["a"]])
                emit("a", lambda a_: a_.copy(sm["b"][:, :], S["cwl"][:, PAD - 1:PAD]), reads=[S["cwl"]], writes=[sm["b"]])
                emit("v", lambda v: v.tensor_sub(out=sm["ct"][:, :], in0=sm["a"][:, :], in1=sm["b"][:, :]),
                     reads=[sm["a"], sm["b"]], writes=[sm["ct"]])
                emit("p", lambda p: matmul_fn(p, psG[:, :], lm_t[:, :], sm["ct"][:, :]),
                     reads=[lm_t, sm["ct"]], writes=[psG])
                emit("v", lambda v: v.tensor_sub(out=sm["C"][:, :], in0=psG[:, :], in1=sm["a"][:, :]),
                     reads=[psG, sm["a"]], writes=[sm["C"]])
                emit("a", lambda a_: a_.activation(S["cw"][:, :], S["cwl"][:, :], F.Identity, bias=sm["C"][:, :]),
                     reads=[S["cwl"], sm["C"]], writes=[S["cw"]])
                # event-masked streams
                emit("v", lambda v: v.tensor_mul(out=S["ew"][:, :], in0=ev[:, :], in1=S["w"][:, :]),
                     reads=[ev, S["w"]], writes=[S["ew"]])
                emit("v", lambda v: v.tensor_mul(out=S["cev"][:, :], in0=ev[:, :], in1=S["cw"][:, :]),
                     reads=[ev, S["cw"]], writes=[S["cev"]])
                # segmented forward scans
                emit("v", lambda v: v.tensor_tensor_scan(out=S["mc"][:, :], data0=cm[:, 0:T], data1=ev[:, :],
                                                         initial=0.0, op0=A.mult, op1=A.add),
                     reads=[cm, ev], writes=[S["mc"]], scan=True)
                emit("v", lambda v: v.tensor_tensor_scan(out=S["sf"][:, :], data0=cm[:, 0:T], data1=S["ew"][:, :],
                                                         initial=0.0, op0=A.mult, op1=A.add),
                     reads=[cm, S["ew"]], writes=[S["sf"]], scan=True)
                emit("v", lambda v: v.tensor_tensor_scan(out=S["fm"][:, :], data0=cm[:, 0:T], data1=S["cev"][:, :],
                                                         initial=0.0, op0=A.mult, op1=A.max),
                     reads=[cm, S["cev"]], writes=[S["fm"]], scan=True)
                # run-end sources + reverse broadcast scans
                emit("v", lambda v: v.tensor_mul(out=S["cev"][:, :], in0=S["mc"][:, :], in1=rsp1[:, :]),
                     reads=[S["mc"], rsp1], writes=[S["cev"]])
                emit("v", lambda v: v.tensor_tensor_scan(out=S["x2"][:, ::-1], data0=cm[:, 1:T + 1][:, ::-1],
                                                         data1=S["cev"][:, ::-1], initial=0.0, op0=A.mult, op1=A.add),
                     reads=[cm, S["cev"]], writes=[S["x2"]], scan=True)  # x2 = Dbc
                emit("v", lambda v: v.tensor_mul(out=S["ew"][:, :], in0=S["sf"][:, :], in1=rsp1[:, :]),
                     reads=[S["sf"], rsp1], writes=[S["ew"]])
                emit("v", lambda v: v.tensor_tensor_scan(out=S["sf"][:, ::-1], data0=cm[:, 1:T + 1][:, ::-1],
                                                         data1=S["ew"][:, ::-1], initial=0.0, op0=A.mult, op1=A.add),
                     reads=[cm, S["ew"]], writes=[S["sf"]], scan=True)  # sf = Sbc
                emit("v", lambda v: v.tensor_mul(out=S["cwl"][:, :], in0=S["fm"][:, :], in1=rsp1[:, :]),
                     reads=[S["fm"], rsp1], writes=[S["cwl"]])
                emit("v", lambda v: v.tensor_tensor_scan(out=S["fm"][:, ::-1], data0=cm[:, 1:T + 1][:, ::-1],
                                                         data1=S["cwl"][:, ::-1], initial=0.0, op0=A.mult, op1=A.add),
                     reads=[cm, S["cwl"]], writes=[S["fm"]], scan=True)  # fm = Rbc
                # per-event Efron term on the valid region
                emit("v", lambda v: v.tensor_sub(out=S["x1"][VS], in0=S["mc"][VS], in1=ev[VS]),
                     reads=[S["mc"], ev], writes=[S["x1"]])  # m
                emit("v", lambda v: v.tensor_scalar_max(S["x3"][VS], S["x2"][VS], 1.0),
                     reads=[S["x2"]], writes=[S["x3"]])  # Dsafe
                emit("v", lambda v: v.reciprocal(out=S["x2"][VS], in_=S["x3"][VS]),
                     reads=[S["x3"]], writes=[S["x2"]])  # recD
                emit("v", lambda v: v.tensor_mul(out=S["x3"][VS], in0=S["x1"][VS], in1=S["x2"][VS]),
                     reads=[S["x1"], S["x2"]], writes=[S["x3"]])  # t1 = m*recD
                emit("v", lambda v: v.tensor_mul(out=S["x1"][VS], in0=S["x3"][VS], in1=S["sf"][VS]),
                     reads=[S["x3"], S["sf"]], writes=[S["x1"]])  # t2 = t1*Sbc
                emit("v", lambda v: v.tensor_sub(out=S["x2"][VS], in0=S["fm"][VS], in1=S["x1"][VS]),
                     reads=[S["fm"], S["x1"]], writes=[S["x2"]])  # arg
                emit("v", lambda v: v.tensor_scalar_max(S["x1"][VS], S["x2"][VS], 1e-30),
                     reads=[S["x2"]], writes=[S["x1"]])  # argc
                emit("a", lambda a_: a_.activation(S["x2"][VS], S["x1"][VS], F.Ln),
                     reads=[S["x1"]], writes=[S["x2"]])  # lsl
                emit("v", lambda v: v.tensor_mul(out=S["x3"][VS], in0=S["x2"][VS], in1=ev[VS]),
                     reads=[S["x2"], ev], writes=[S["x3"]])
                emit("v", lambda v: v.tensor_reduce(out=pp_t[:, 0:1], in_=S["x3"][VS], axis=mybir.AxisListType.X, op=A.add),
                     reads=[S["x3"]], writes=[pp_t])
                # per-column combine
                emit("p", lambda p: matmul_fn(p, ps2[0:CPP, :], bm_t[:, :], pp_t[:, :]),
                     reads=[bm_t, pp_t], writes=[ps2])
                emit("a", lambda a_: a_.copy(cs_t[0:CPP, :], ps2[0:CPP, :]), reads=[ps2], writes=[cs_t])
                emit("v", lambda v: v.tensor_sub(out=sm["a"][0:CPP, :], in0=cs_t[0:CPP, 0:1], in1=cs_t[0:CPP, 1:2]),
                     reads=[cs_t], writes=[sm["a"]])
                emit("v", lambda v: v.reciprocal(out=sm["b"][0:CPP, :], in_=cs_t[0:CPP, 2:3]),
                     reads=[cs_t], writes=[sm["b"]])
                emit("v", lambda v: v.tensor_mul(out=loss_t[0:CPP, :], in0=sm["a"][0:CPP, :], in1=sm["b"][0:CPP, :]),
                     reads=[sm["a"], sm["b"]], writes=[loss_t])
                emit_dma("dout", ls_d[g * CPP:(g + 1) * CPP], loss_t[0:CPP, :], reads=[loss_t])

            def fin(proxy):
                proxy.wait_ge(sems["dout"], 16 * cnt["dout"])

            blk.sync(fin)
    return nc


def kernel(logh, events, durations):
    lh_p, du_p, ev_p, lmat, bmat = _host_prep(logh, events, durations)
    if "nc" not in _CACHE:
        _CACHE["nc"] = _build_bass()
    from concourse.bass_utils import run_bass_kernel_spmd
    in_maps = []
    for m in range(NCORES):
        sl = slice(m * CPC, (m + 1) * CPC)
        in_maps.append({"lh": lh_p[sl], "du": du_p[sl], "ev": ev_p[sl],
                        "lmat": lmat, "bmat": bmat})
    res = run_bass_kernel_spmd(_CACHE["nc"], in_maps, list(range(NCORES)))
    lt = np.concatenate([res.results[m]["loss"] for m in range(NCORES)]).astype(np.float32)
    li = lt > 0
    return np.float32(np.sum(np.where(li, lt, np.float32(0.0)), dtype=np.float32) / np.float32(li.sum()))


if __name__ == "__main__":
    rng = np.random.default_rng(0)
    logh = rng.standard_normal((B, N, E)).astype(np.float32)
    events = rng.integers(0, 2, (B, N, E)).astype(np.int32)
    durations = rng.integers(0, 1000, (B, N, E)).astype(np.int32)
    print("kernel:", kernel(logh, events, durations))



# revision 2
# speedup vs baseline: 32.7034x; 32.7034x over previous
"""CoxPHLoss (Efron ties) Trainium2 kernel — v2.

Host does layout only: per-column stable sort by descending duration
(index-space), sentinel padding, and the 0/1 run-boundary mask
cm[t] = (du[t]==du[t-1]) (index-space equality). All FP loss arithmetic
runs on 8 NeuronCores, single pass over [128, T] tiles (128 partitions
= 16 columns x 8 chunks of 4096):
  exp -> cumsum scan (+ PE carry fixup) -> 5 segmented scans keyed on cm
  (fwd/rev suffix forms) -> division-free Efron term
  ln(D*R - m*S) - ln(D) -> masked reductions -> per-column losses via
  PE combine. Final masked mean over 128 column losses on host.
"""
import sys

sys.path.insert(0, "/opt/trn_rl_repo")

import numpy as np

B, N, E = 16, 32768, 8
NCORES = 8
COLS = B * E              # 128 independent (b, i) columns
CPC = COLS // NCORES      # 16 columns per core
PAD = 128                 # > max run length of equal durations in a column
CH = 8                    # chunks per column
V = N // CH               # 4096 valid samples per chunk
T = V + 2 * PAD           # 4352 tile width
PV = PAD + V              # forward scans cover [0, PV); reverse scans [PAD, T)
L = N + 2 * PAD           # 33024 padded column length

_CACHE = {}


def _host_prep(logh, events, durations):
    lh = np.ascontiguousarray(logh.transpose(0, 2, 1).reshape(COLS, N))
    ev = np.ascontiguousarray(events.transpose(0, 2, 1).reshape(COLS, N))
    du = np.ascontiguousarray(durations.transpose(0, 2, 1).reshape(COLS, N))
    order = np.argsort(-du, axis=1, kind="stable")
    lh_s = np.take_along_axis(lh, order, 1).astype(np.float32)
    ev_s = np.take_along_axis(ev, order, 1).astype(np.float32)
    du_s = np.take_along_axis(du, order, 1)

    lh_p = np.zeros((COLS, L), np.float32)
    ev_p = np.zeros((COLS, L), np.float32)
    du_p = np.empty((COLS, L), np.int64)
    du_p[:, :PAD] = -2
    du_p[:, PAD + N:] = -1
    lh_p[:, PAD:PAD + N] = lh_s
    ev_p[:, PAD:PAD + N] = ev_s
    du_p[:, PAD:PAD + N] = du_s

    cm_p = np.zeros((COLS, L + 1), np.float32)
    cm_p[:, 1:L] = (du_p[:, 1:] == du_p[:, :-1]).astype(np.float32)

    lmat = np.zeros((128, 128), np.float32)   # G[p] = sum_{k<=p, same col} ct[k]
    for p in range(128):
        c0 = (p // CH) * CH
        lmat[c0:p + 1, p] = 1.0
    bmat = np.zeros((128, CPC), np.float32)   # colsum[m] = sum over col m's chunks
    for k in range(128):
        bmat[k, k // CH] = 1.0
    return lh_p, cm_p, ev_p, lmat, bmat


def _build_bass(reps=1):
    import concourse.bass as bass
    from concourse import mybir
    import contextlib

    A = mybir.AluOpType
    F = mybir.ActivationFunctionType
    f32 = mybir.dt.float32
    nc = bass.Bass()

    lh_d = nc.dram_tensor("lh", [CPC, L], f32, kind="ExternalInput")
    cm_d = nc.dram_tensor("cm", [CPC, L + 1], f32, kind="ExternalInput")
    ev_d = nc.dram_tensor("ev", [CPC, L], f32, kind="ExternalInput")
    lm_d = nc.dram_tensor("lmat", [128, 128], f32, kind="ExternalInput")
    bm_d = nc.dram_tensor("bmat", [128, CPC], f32, kind="ExternalInput")
    ls_d = nc.dram_tensor("loss", [CPC], f32, kind="ExternalOutput")

    st = contextlib.ExitStack()

    def sb(shape, name):
        return st.enter_context(nc.sbuf_tensor(name, shape, f32))

    # slabs; roles change over the pipeline (see comments inline)
    bA = sb([128, T], "bA")      # lh -> cw -> D -> targ
    bB = sb([128, T], "bB")      # cev -> t1 -> lsl
    bC = sb([128, T], "bC")      # ev
    bM = sb([128, T + 1], "bM")  # cm -> m -> relu(targ)
    bW = sb([128, T], "bW")      # w -> mc -> ldd
    bX = sb([128, T], "bX")      # cwl -> ew
    bS1 = sb([128, T], "bS1")    # p1 dump -> dsf -> u
    bS2 = sb([128, T], "bS2")    # sfw -> S
    bS3 = sb([128, T], "bS3")    # ssf -> q2 -> racc dump
    bS4 = sb([128, T], "bS4")    # rbc -> relu(D-1) -> diff
    lm_t = sb([128, 128], "lm_t")
    bm_t = sb([128, CPC], "bm_t")
    sm = {n: sb([128, 1], n) for n in ["sa", "sb_", "ct", "sC", "ones", "trash", "neg1", "eps"]}
    cs_t = sb([128, 3], "cs_t")
    pp_t = sb([128, 3], "pp_t")
    loss_t = sb([128, 1], "loss_t")
    psG = st.enter_context(nc.psum_tensor("psG", [128, 1], f32))
    ps2 = st.enter_context(nc.psum_tensor("ps2", [128, 3], f32))

    sems = {n: st.enter_context(nc.semaphore(n))
            for n in ["sv", "sa", "sp", "sg", "dlh", "dcm", "dev", "dlm", "dbm", "dout"]}

    with st:
        with nc.Block() as blk:
            eng_of = {"v": "vector", "a": "scalar", "p": "tensor", "g": "gpsimd"}
            sem_of = {"v": "sv", "a": "sa", "p": "sp", "g": "sg"}
            cnt = {"v": 0, "a": 0, "p": 0, "g": 0, "dlh": 0, "dcm": 0, "dev": 0, "dlm": 0, "dbm": 0, "dout": 0}
            waited = {}
            track = {}  # id(handle) -> {"w": [(eng, tick)...], "r": [(eng, tick)...]}

            def rec(h):
                return track.setdefault(id(h), {"w": [], "r": []})

            def dep_waits(eng, reads, writes):
                need = {}
                for h in reads:
                    for k, t in rec(h)["w"]:
                        need[k] = max(need.get(k, 0), t)
                for h in writes:
                    r = rec(h)
                    for k, t in r["w"] + r["r"]:
                        need[k] = max(need.get(k, 0), t)
                out = []
                for k, t in need.items():
                    semname = k if k.startswith("d") else sem_of[k]
                    val = t * 16 if k.startswith("d") else t
                    if waited.get((eng, semname), -1) < val:
                        out.append((semname, val))
                        waited[(eng, semname)] = val
                return out

            def note(eng, tick, reads, writes):
                for h in reads:
                    rec(h)["r"].append((eng, tick))
                for h in writes:
                    r = rec(h)
                    r["w"].append((eng, tick))
                    r["r"] = []

            def emit(eng, fn, reads=(), writes=(), scan=False):
                ws = dep_waits(eng, reads, writes)
                tick = cnt[eng] + 1

                def body(proxy):
                    for semname, val in ws:
                        proxy.wait_ge(sems[semname], val)
                    fn(proxy).then_inc(sems[sem_of[eng]], 1)

                getattr(blk, eng_of[eng])(body)
                cnt[eng] = tick
                note(eng, tick, reads, writes)

            def emit_dma(semname, out_ap, in_ap, reads=(), writes=()):
                ws = dep_waits(semname, reads, writes)
                cnt[semname] += 1
                tick = cnt[semname]

                def body(proxy):
                    for sn, val in ws:
                        proxy.wait_ge(sems[sn], val)
                    proxy.dma_start(out=out_ap, in_=in_ap).then_inc(sems[semname], 16)

                blk.sync(body)
                note(semname, tick, reads, writes)

            def matmul_fn(proxy, out, lhsT, rhs):
                try:
                    return proxy.matmul(out, lhsT, rhs, start=True, stop=True)
                except TypeError:
                    return proxy.matmul(contextlib.ExitStack(), out, lhsT, rhs, start=True, stop=True)

            emit_dma("dlm", lm_t[:, :], lm_d[:, :], writes=[lm_t])
            emit_dma("dbm", bm_t[:, :], bm_d[:, :], writes=[bm_t])
            emit("v", lambda v: v.memset(sm["ones"][:, :], 1.0), writes=[sm["ones"]])
            emit("v", lambda v: v.memset(sm["neg1"][:, :], -1.0), writes=[sm["neg1"]])
            emit("v", lambda v: v.memset(sm["eps"][:, :], 1e-30), writes=[sm["eps"]])

            VS = np.s_[:, PAD:PV]
            ones_T = sm["ones"][:, :].broadcast_to([128, T])

            for _ in range(reps):
                # ---- input DMAs ----
                emit_dma("dlh", bA[:, :],
                         bass.AP(tensor=lh_d[:, :].tensor, offset=0, ap=[[L, CPC], [V, CH], [1, T]]),
                         writes=[bA])
                emit_dma("dcm", bM[:, :],
                         bass.AP(tensor=cm_d[:, :].tensor, offset=0, ap=[[L + 1, CPC], [V, CH], [1, T + 1]]),
                         writes=[bM])
                emit_dma("dev", bC[:, :],
                         bass.AP(tensor=ev_d[:, :].tensor, offset=0, ap=[[L, CPC], [V, CH], [1, T]]),
                         writes=[bC])
                lh, cm, ev = bA, bM, bC

                # ---- early masked reductions (off critical path) ----
                emit("v", lambda v: v.scalar_tensor_tensor(
                    out=bS1[VS], in0=ev[VS], scalar=1.0, in1=lh[VS],
                    op0=A.mult, op1=A.mult, accum_out=pp_t[:, 1:2]),
                    reads=[ev, lh], writes=[pp_t, bS1])
                emit("v", lambda v: v.tensor_reduce(out=pp_t[:, 2:3], in_=ev[VS],
                                                    axis=mybir.AxisListType.X, op=A.add),
                     reads=[ev], writes=[pp_t])

                # ---- w -> cwl -> carry fixup -> cw -> cev ----
                emit("a", lambda a_: a_.activation(bW[:, :], lh[:, :], F.Exp), reads=[lh], writes=[bW])
                emit("v", lambda v: v.tensor_tensor_scan(out=bX[:, :], data0=ones_T,
                                                         data1=bW[:, :], initial=0.0, op0=A.mult, op1=A.add),
                     reads=[bW, sm["ones"]], writes=[bX], scan=True)
                emit("a", lambda a_: a_.copy(sm["sa"][:, :], bX[:, PV - 1:PV]), reads=[bX], writes=[sm["sa"]])
                emit("a", lambda a_: a_.copy(sm["sb_"][:, :], bX[:, PAD - 1:PAD]), reads=[bX], writes=[sm["sb_"]])
                emit("v", lambda v: v.tensor_sub(out=sm["ct"][:, :], in0=sm["sa"][:, :], in1=sm["sb_"][:, :]),
                     reads=[sm["sa"], sm["sb_"]], writes=[sm["ct"]])
                emit("p", lambda p: matmul_fn(p, psG[:, :], lm_t[:, :], sm["ct"][:, :]),
                     reads=[lm_t, sm["ct"]], writes=[psG])
                emit("v", lambda v: v.tensor_sub(out=sm["sC"][:, :], in0=psG[:, :], in1=sm["sa"][:, :]),
                     reads=[psG, sm["sa"]], writes=[sm["sC"]])
                emit("a", lambda a_: a_.activation(bA[:, :], bX[:, :], F.Identity, bias=sm["sC"][:, :]),
                     reads=[bX, sm["sC"]], writes=[bA])
                cw = bA
                # cev = ev*cw on DVE (critical path to rbc); bB is free
                emit("v", lambda v: v.scalar_tensor_tensor(out=bB[:, PAD:T], in0=ev[:, PAD:T], scalar=1.0,
                                                           in1=cw[:, PAD:T], op0=A.mult, op1=A.mult),
                     reads=[ev, cw], writes=[bB])
                cev = bB
                # ew = ev*w on gpsimd (off critical path)
                emit("g", lambda g: g.tensor_tensor(out=bX[:, :], in0=ev[:, :], in1=bW[:, :], op=A.mult),
                     reads=[ev, bW], writes=[bX])
                ew = bX

                # ---- segmented scans ----
                emit("v", lambda v: v.tensor_tensor_scan(out=bS4[:, PAD:T][:, ::-1],
                                                         data0=cm[:, PAD + 1:T + 1][:, ::-1],
                                                         data1=cev[:, PAD:T][:, ::-1],
                                                         initial=0.0, op0=A.mult, op1=A.max),
                     reads=[cm, cev], writes=[bS4], scan=True)
                rbc = bS4
                emit("v", lambda v: v.tensor_tensor_scan(out=bS1[:, PAD:T][:, ::-1],
                                                         data0=cm[:, PAD + 1:T + 1][:, ::-1],
                                                         data1=ev[:, PAD:T][:, ::-1],
                                                         initial=0.0, op0=A.mult, op1=A.add),
                     reads=[cm, ev], writes=[bS1], scan=True)
                dsf = bS1
                # mc -> bW (waits for gpsimd ew to free w; dsf above hides that wait)
                emit("v", lambda v: v.tensor_tensor_scan(out=bW[:, 0:PV], data0=cm[:, 0:PV], data1=ev[:, 0:PV],
                                                         initial=0.0, op0=A.mult, op1=A.add),
                     reads=[cm, ev], writes=[bW], scan=True)
                mc = bW
                emit("v", lambda v: v.tensor_tensor_scan(out=bS2[:, 0:PV], data0=cm[:, 0:PV], data1=ew[:, 0:PV],
                                                         initial=0.0, op0=A.mult, op1=A.add),
                     reads=[cm, ew], writes=[bS2], scan=True)
                sfw = bS2
                emit("v", lambda v: v.tensor_tensor_scan(out=bS3[:, PAD:T][:, ::-1],
                                                         data0=cm[:, PAD + 1:T + 1][:, ::-1],
                                                         data1=ew[:, PAD:T][:, ::-1],
                                                         initial=0.0, op0=A.mult, op1=A.add),
                     reads=[cm, ew], writes=[bS3], scan=True)
                ssf = bS3

                # ---- epilogue on the valid slice ----
                # m = mc - ev -> bM (cm dead after scans)
                emit("v", lambda v: v.scalar_tensor_tensor(out=bM[VS], in0=ev[VS], scalar=-1.0,
                                                           in1=mc[VS], op0=A.mult, op1=A.add),
                     reads=[ev, mc], writes=[bM])
                m_ = bM
                # D = m + dsf -> bA (cw dead after cev)
                emit("v", lambda v: v.scalar_tensor_tensor(out=bA[VS], in0=m_[VS], scalar=1.0,
                                                           in1=dsf[VS], op0=A.mult, op1=A.add),
                     reads=[m_, dsf], writes=[bA])
                D_ = bA
                # t1 = max(D,1)*rbc -> bB (cev dead after rbc)
                emit("v", lambda v: v.scalar_tensor_tensor(out=bB[VS], in0=D_[VS], scalar=1.0,
                                                           in1=rbc[VS], op0=A.max, op1=A.mult),
                     reads=[D_, rbc], writes=[bB])
                t1 = bB
                # ldd = ln(max(D,1)) on Act: relu(D-1) -> bS4 (rbc dead), ln(x+1) -> bW (mc dead)
                emit("a", lambda a_: a_.activation(bS4[VS], D_[VS], F.Relu, bias=sm["neg1"][:, :]),
                     reads=[D_, sm["neg1"]], writes=[bS4])
                Dr = bS4
                emit("a", lambda a_: a_.activation(bW[VS], Dr[VS], F.Ln, bias=sm["ones"][:, :]),
                     reads=[Dr, sm["ones"]], writes=[bW])
                ldd = bW
                # u = sfw + ssf -> bS1 (dsf dead after D)
                emit("v", lambda v: v.scalar_tensor_tensor(out=bS1[VS], in0=sfw[VS], scalar=1.0,
                                                           in1=ssf[VS], op0=A.mult, op1=A.add),
                     reads=[sfw, ssf], writes=[bS1])
                u_ = bS1
                # S = u - ew -> bS2 (sfw dead after u)
                emit("v", lambda v: v.scalar_tensor_tensor(out=bS2[VS], in0=ew[VS], scalar=-1.0,
                                                           in1=u_[VS], op0=A.mult, op1=A.add),
                     reads=[ew, u_], writes=[bS2])
                S_ = bS2
                # q2 = m*S -> bS3 (ssf dead after u)
                emit("v", lambda v: v.scalar_tensor_tensor(out=bS3[VS], in0=m_[VS], scalar=1.0,
                                                           in1=S_[VS], op0=A.mult, op1=A.mult),
                     reads=[m_, S_], writes=[bS3])
                q2 = bS3
                # targ = t1 - q2 -> bA (D dead after t1 and relu)
                emit("v", lambda v: v.scalar_tensor_tensor(out=bA[VS], in0=q2[VS], scalar=-1.0,
                                                           in1=t1[VS], op0=A.mult, op1=A.add),
                     reads=[q2, t1], writes=[bA])
                targ = bA
                # tr = relu(targ) -> bM (m dead after q2)
                emit("a", lambda a_: a_.activation(bM[VS], targ[VS], F.Relu), reads=[targ], writes=[bM])
                tr = bM
                # lsl = ln(tr + 1e-30) -> bB (t1 dead after targ)
                emit("a", lambda a_: a_.activation(bB[VS], tr[VS], F.Ln, bias=sm["eps"][:, :]), reads=[tr, sm["eps"]], writes=[bB])
                lsl = bB
                # diff = lsl - ldd -> bS4 (Dr dead after ldd)
                emit("v", lambda v: v.scalar_tensor_tensor(out=bS4[VS], in0=ldd[VS], scalar=-1.0,
                                                           in1=lsl[VS], op0=A.mult, op1=A.add),
                     reads=[ldd, lsl], writes=[bS4])
                diff = bS4
                # pp0 = sum ev*diff (dump -> bS3; q2 dead after targ)
                emit("v", lambda v: v.scalar_tensor_tensor(
                    out=bS3[VS], in0=ev[VS], scalar=1.0, in1=diff[VS],
                    op0=A.mult, op1=A.mult, accum_out=pp_t[:, 0:1]),
                    reads=[ev, diff], writes=[pp_t, bS3])

                # ---- per-column combine ----
                emit("p", lambda p: matmul_fn(p, ps2[0:CPC, :], bm_t[:, :], pp_t[:, :]),
                     reads=[bm_t, pp_t], writes=[ps2])
                emit("a", lambda a_: a_.copy(cs_t[0:CPC, :], ps2[0:CPC, :]), reads=[ps2], writes=[cs_t])
                emit("v", lambda v: v.tensor_sub(out=sm["sa"][0:CPC, :], in0=cs_t[0:CPC, 0:1], in1=cs_t[0:CPC, 1:2]),
                     reads=[cs_t], writes=[sm["sa"]])
                emit("v", lambda v: v.reciprocal(out=sm["sb_"][0:CPC, :], in_=cs_t[0:CPC, 2:3]),
                     reads=[cs_t], writes=[sm["sb_"]])
                emit("v", lambda v: v.tensor_mul(out=loss_t[0:CPC, :], in0=sm["sa"][0:CPC, :], in1=sm["sb_"][0:CPC, :]),
                     reads=[sm["sa"], sm["sb_"]], writes=[loss_t])
                emit_dma("dout", ls_d[0:CPC], loss_t[0:CPC, :], reads=[loss_t])

            def fin(proxy):
                proxy.wait_ge(sems["dout"], 16 * cnt["dout"])

            blk.sync(fin)
    return nc


def kernel(logh, events, durations):
    lh_p, cm_p, ev_p, lmat, bmat = _host_prep(logh, events, durations)
    if "nc" not in _CACHE:
        _CACHE["nc"] = _build_bass()
    from concourse.bass_utils import run_bass_kernel_spmd
    in_maps = []
    for m in range(NCORES):
        sl = slice(m * CPC, (m + 1) * CPC)
        in_maps.append({"lh": lh_p[sl], "cm": cm_p[sl], "ev": ev_p[sl],
                        "lmat": lmat, "bmat": bmat})
    res = run_bass_kernel_spmd(_CACHE["nc"], in_maps, list(range(NCORES)))
    lt = np.concatenate([res.results[m]["loss"] for m in range(NCORES)]).astype(np.float32)
    li = lt > 0
    return np.float32(np.sum(np.where(li, lt, np.float32(0.0)), dtype=np.float32) / np.float32(li.sum()))


if __name__ == "__main__":
    rng = np.random.default_rng(0)
    logh = rng.standard_normal((B, N, E)).astype(np.float32)
    events = rng.integers(0, 2, (B, N, E)).astype(np.int32)
    durations = rng.integers(0, 1000, (B, N, E)).astype(np.int32)
    print("kernel:", kernel(logh, events, durations))
